# revision 3
# baseline (speedup 1.0000x reference)
"""Sparse 3x3x3 conv (C_in=C_out=1) over N=2M voxels in a 256^3 grid.

Strategy (dense_cnn, v2 "zx-pack"): densify into a zero-padded 258^3
volume [z, x, y].  Pack a 16z x 8x input window into the 128 SBUF
partitions (partition p = x_i*16 + z_i, each holding a 258-long y-line);
a single 128x84 "band" matrix then applies all 9 (dz,dx) taps at once,
producing a 14z x 6x = 84-row output tile.  Only the 3 dy taps need
separate matmuls (free-dim y shifts), PSUM-accumulated.  Per output
tile: 3 matmuls instead of the 10 of the z-banded scheme.  x is sharded
across 8 cores (32 output x-rows each); the host pre-packs the SBUF
image and unpacks the tiled output, so all device DMA is contiguous.
"""

import numpy as np

import concourse.bass as bass
import concourse.mybir as mybir
import concourse.tile as tile
from concourse import bacc
from concourse.bass_utils import run_bass_kernel_spmd

G = 256              # grid extent
P = G + 2            # padded extent
NCORES = 8
XS = G // NCORES     # 32 output x-rows per core
XWIN = XS + 2        # 34-col input window (x halo)

ZT, XT = 14, 6       # output tile extent (z, x)
ZI, XI = 16, 8       # input window extent (z, x) -> 128 partitions
M = ZT * XT          # 84 output partitions per tile
NZT = -(-G // ZT)    # 19 z-tiles (last partial: 4 valid rows)
NXT = XS // XT + 1   # 6 x-tiles (last partial: 2 valid rows)
NXP = NXT // 2       # 3 x-pairs (free dim packs 2 x-tiles per matmul)
LINE = XT * P        # 1548 elements per (partition, z-tile)
OZ = ZT * NZT        # 266 padded output z rows
OX = XT * NXT        # 36 padded output x rows

PE_DT = mybir.dt.float16
NP_DT = np.float16


def _build_nc(iters=1):
    nc = bacc.Bacc("TRN2", target_bir_lowering=False, debug=False)
    pk = nc.dram_tensor("pk", [NZT, 128, LINE], PE_DT, kind="ExternalInput")
    wb = nc.dram_tensor("wb", [128, 3, M], PE_DT, kind="ExternalInput")
    out = nc.dram_tensor("out", [NZT * NXP, M, 2 * G], PE_DT,
                         kind="ExternalOutput")

    with tile.TileContext(nc) as tc:
        with (
            tc.tile_pool(name="w", bufs=1) as wp,
            tc.tile_pool(name="inp", bufs=4) as ip,
            tc.tile_pool(name="ps", bufs=8, space="PSUM") as pp,
            tc.tile_pool(name="ob", bufs=6) as op,
        ):
            wt = wp.tile([128, 3, M], PE_DT)
            nc.sync.dma_start(out=wt[:], in_=wb[:])

            def body(_i=None):
                for zt in range(NZT):
                    it = ip.tile([128, XT, P], PE_DT, tag="inp", name="it")
                    nc.sync.dma_start(
                        out=it.rearrange("p a b -> p (a b)"), in_=pk[zt])
                    pss = []
                    for xp in range(NXP):
                        ps = pp.tile([M, 2, G], mybir.dt.float32, tag="ps",
                                     name="ps")
                        pss.append(ps)
                    for di, dy in enumerate((-1, 0, 1)):
                        for xp in range(NXP):
                            nc.tensor.matmul(
                                pss[xp][:],
                                wt[:, di, :],
                                it[:, 2 * xp:2 * xp + 2, 1 + dy:1 + dy + G],
                                start=(di == 0),
                                stop=(di == 2),
                            )
                    for xp in range(NXP):
                        sb = op.tile([M, 2 * G], PE_DT, tag="ob", name="sb")
                        if xp % 2 == 0:
                            nc.scalar.copy(out=sb[:], in_=pss[xp].rearrange(
                                "m a b -> m (a b)"))
                        else:
                            nc.vector.tensor_copy(sb[:], pss[xp].rearrange(
                                "m a b -> m (a b)"))
                        nc.scalar.dma_start(out=out[zt * NXP + xp], in_=sb[:])

            if iters == 1:
                body()
            else:
                with tc.For_i(0, iters, 1):
                    body(0)
    nc.finalize()
    return nc


_NC_CACHE = {}


def _get_nc(iters=1):
    if iters not in _NC_CACHE:
        _NC_CACHE[iters] = _build_nc(iters)
    return _NC_CACHE[iters]


def _make_bands(W):
    W27 = np.asarray(W, dtype=np.float32).reshape(27)
    wb = np.zeros((128, 3, M), dtype=np.float32)
    for x_i in range(XI):
        for z_i in range(ZI):
            p = x_i * ZI + z_i
            for x_o in range(XT):
                dxi = x_i - x_o
                if not (0 <= dxi < 3):
                    continue
                for z_o in range(ZT):
                    dzi = z_i - z_o
                    if not (0 <= dzi < 3):
                        continue
                    m = x_o * ZT + z_o
                    for di in range(3):
                        wb[p, di, m] = W27[dxi * 9 + di * 3 + dzi]
    return wb.astype(NP_DT)


def _make_in_maps(coords, feats, W):
    coords = np.asarray(coords)
    x = coords[:, 0].astype(np.int64)
    y = coords[:, 1].astype(np.int64)
    z = coords[:, 2].astype(np.int64)
    Dp = np.zeros((P, P, P), dtype=NP_DT)          # [z_pad, x_pad, y_pad]
    # reversed order: on (unexpected) duplicate coords the first occurrence
    # wins, matching the reference's stable argsort + searchsorted lookup
    Dp[z[::-1] + 1, x[::-1] + 1, y[::-1] + 1] = \
        np.asarray(feats)[::-1, 0].astype(NP_DT)
    wb = _make_bands(W)
    in_maps = []
    for c in range(NCORES):
        pkc = np.zeros((NZT, 128, XT, P), dtype=NP_DT)
        gx0 = XS * c                               # padded-x base of window
        for x_i in range(XI):
            nx = 1 + (XWIN - 1 - x_i) // XT        # valid x-tiles
            nx = min(nx, NXT)
            for z_i in range(ZI):
                p = x_i * ZI + z_i
                nz = 1 + (P - 1 - z_i) // ZT       # valid z-tiles
                nz = min(nz, NZT)
                pkc[:nz, p, :nx, :] = Dp[
                    z_i:z_i + ZT * nz:ZT,
                    gx0 + x_i:gx0 + x_i + XT * nx:XT,
                    :,
                ]
        in_maps.append({"pk": pkc.reshape(NZT, 128, LINE), "wb": wb})
    return in_maps, x, y, z


def kernel(coords, feats, W):
    in_maps, x, y, z = _make_in_maps(coords, feats, W)
    nc = _get_nc(1)
    res = run_bass_kernel_spmd(nc, in_maps, list(range(NCORES)))
    # out[zt*NXP+xp, m=(x_o*ZT+z_o), (x_t, y)] -> [z, x, y]
    parts = []
    for c in range(NCORES):
        o = res.results[c]["out"].reshape(NZT, NXP, XT, ZT, 2, G)
        # dims: zt, xp, x_o, z_o, x_t, y  ->  z = zt*ZT+z_o, x = (xp*2+x_t)*XT+x_o
        o = o.transpose(0, 3, 1, 4, 2, 5).reshape(OZ, OX, G)
        parts.append(o[:G, :XS, :])
    Ofull = np.concatenate(parts, axis=1)          # [z, x, y]
    return Ofull[z, x, y].astype(np.float32).reshape(-1, 1)


# revision 9
# speedup vs baseline: 1.9088x; 1.9088x over previous
"""Sparse 3x3x3 conv (C_in=C_out=1) over N=2M voxels in a 256^3 grid.

Strategy (dense_cnn, "zx-pack"): densify into a zero-padded 258^3
volume [z, x, y].  Pack a (ZT+2) x (XT+2) (z,x) input window into the
128 SBUF partitions (partition p = x_i*(ZT+2) + z_i, each holding a
258-long y-line); a single 128 x (ZT*XT) "band" matrix then applies all
9 (dz,dx) taps at once.  Only the 3 dy taps need separate matmuls
(free-dim y shifts), PSUM-accumulated; the free dim spans all XT
x-tiles of a z-tile (N = XT*256).  Per z-tile: 3 matmuls.  x is sharded
across 8 cores (32 output x-rows each); the host pre-packs the SBUF
image and unpacks the tiled output, so all device DMA is contiguous.
"""

import numpy as np

import concourse.bass as bass
import concourse.mybir as mybir
import concourse.tile as tile
from concourse import bacc
from concourse.bass_utils import run_bass_kernel_spmd

G = 256              # grid extent
P = G + 2            # padded extent
NCORES = 8
XS = G // NCORES     # 32 output x-rows per core
XWIN = XS + 2        # 34-col input window (x halo)

ZT, XT = 10, 8       # output tile extent (z, x)
ZI, XI = ZT + 2, XT + 2
NPART = ZI * XI      # 120 used partitions
M = ZT * XT          # 80 output partitions per tile
NZT = -(-G // ZT)    # 26 z-tiles (last partial)
NXT = -(-XS // XT)   # 4 x-tiles (exact)
LINE = NXT * P       # elements per (partition, z-tile)
OZ = ZT * NZT        # padded output z rows
OX = XT * NXT        # = 32 output x rows (exact)

PE_DT = mybir.dt.float16
NP_DT = np.float16


def _build_nc(iters=1, do_in=True, do_out=True, ibufs=10, obufs=8, quad=False):
    nc = bacc.Bacc("TRN2", target_bir_lowering=False, debug=False)
    pk = nc.dram_tensor("pk", [NZT, 128, LINE], PE_DT, kind="ExternalInput")
    wb = nc.dram_tensor("wb", [128, 3, M], PE_DT, kind="ExternalInput")
    out = nc.dram_tensor("out", [NZT, M, NXT * G], PE_DT,
                         kind="ExternalOutput")
    NXP = NXT // 2

    with tile.TileContext(nc) as tc:
        with (
            tc.tile_pool(name="w", bufs=1) as wp,
            tc.tile_pool(name="inp", bufs=NZT if not do_in else ibufs) as ip,
            tc.tile_pool(name="ps", bufs=4 if quad else 8, space="PSUM") as pp,
            tc.tile_pool(name="ob", bufs=obufs) as op,
        ):
            wt = wp.tile([128, 3, M], PE_DT)
            nc.sync.dma_start(out=wt[:], in_=wb[:])

            def load_tile(zt):
                it = ip.tile([128, NXT, P], PE_DT, tag="inp", name="it")
                nc.sync.dma_start(
                    out=it.rearrange("p a b -> p (a b)"), in_=pk[zt])
                return it

            hoisted = None
            if not do_in:
                hoisted = [load_tile(zt) for zt in range(NZT)]

            def body(_i=None):
                for zt in range(NZT):
                    it = hoisted[zt] if hoisted else load_tile(zt)
                    emit = do_out or zt == NZT - 1
                    if quad:
                        ps = pp.tile([M, NXT, G], mybir.dt.float32, tag="ps",
                                     name="ps")
                        for di in range(3):
                            nc.tensor.matmul(
                                ps[:],
                                wt[:, di, :],
                                it[:, :, di:di + G],
                                start=(di == 0),
                                stop=(di == 2),
                            )
                        if emit:
                            sb = op.tile([M, NXT * G], PE_DT, tag="ob",
                                         name="sb")
                            nc.vector.tensor_copy(sb[:], ps.rearrange(
                                "m a b -> m (a b)"))
                            nc.scalar.dma_start(out=out[zt], in_=sb[:])
                    else:
                        pss = [pp.tile([M, 2, G], mybir.dt.float32, tag="ps",
                                       name="ps") for _ in range(NXP)]
                        for xp in range(NXP):
                            for di in range(3):
                                nc.tensor.matmul(
                                    pss[xp][:],
                                    wt[:, di, :],
                                    it[:, 2 * xp:2 * xp + 2, di:di + G],
                                    start=(di == 0),
                                    stop=(di == 2),
                                )
                        if emit:
                            sb = op.tile([M, NXT, G], PE_DT, tag="ob",
                                         name="sb")
                            for xp in range(NXP):
                                nc.vector.tensor_copy(
                                    sb[:, 2 * xp:2 * xp + 2, :], pss[xp][:])
                            nc.scalar.dma_start(
                                out=out[zt],
                                in_=sb.rearrange("m a b -> m (a b)"))

            if iters == 1:
                body()
            else:
                with tc.For_i(0, iters, 1):
                    body(0)
    nc.finalize()
    return nc


_NC_CACHE = {}


def _get_nc(iters=1, **kw):
    key = (iters, tuple(sorted(kw.items())))
    if key not in _NC_CACHE:
        _NC_CACHE[key] = _build_nc(iters, **kw)
    return _NC_CACHE[key]


def _make_bands(W):
    W27 = np.asarray(W, dtype=np.float32).reshape(27)
    wb = np.zeros((128, 3, M), dtype=np.float32)
    for x_i in range(XI):
        for z_i in range(ZI):
            p = x_i * ZI + z_i
            for x_o in range(XT):
                dxi = x_i - x_o
                if not (0 <= dxi < 3):
                    continue
                for z_o in range(ZT):
                    dzi = z_i - z_o
                    if not (0 <= dzi < 3):
                        continue
                    m = x_o * ZT + z_o
                    for di in range(3):
                        wb[p, di, m] = W27[dxi * 9 + di * 3 + dzi]
    return wb.astype(NP_DT)


def _make_in_maps(coords, feats, W):
    coords = np.asarray(coords)
    x = coords[:, 0].astype(np.int64)
    y = coords[:, 1].astype(np.int64)
    z = coords[:, 2].astype(np.int64)
    Dp = np.zeros((P, P, P), dtype=NP_DT)          # [z_pad, x_pad, y_pad]
    # reversed order: on (unexpected) duplicate coords the first occurrence
    # wins, matching the reference's stable argsort + searchsorted lookup
    Dp[z[::-1] + 1, x[::-1] + 1, y[::-1] + 1] = \
        np.asarray(feats)[::-1, 0].astype(NP_DT)
    wb = _make_bands(W)
    in_maps = []
    for c in range(NCORES):
        pkc = np.zeros((NZT, 128, NXT, P), dtype=NP_DT)
        gx0 = XS * c                               # padded-x base of window
        for x_i in range(XI):
            nx = min(1 + (XWIN - 1 - x_i) // XT, NXT)
            for z_i in range(ZI):
                p = x_i * ZI + z_i
                nz = min(1 + (P - 1 - z_i) // ZT, NZT)
                pkc[:nz, p, :nx, :] = Dp[
                    z_i:z_i + ZT * nz:ZT,
                    gx0 + x_i:gx0 + x_i + XT * nx:XT,
                    :,
                ]
        in_maps.append({"pk": pkc.reshape(NZT, 128, LINE), "wb": wb})
    return in_maps, x, y, z


def kernel(coords, feats, W):
    in_maps, x, y, z = _make_in_maps(coords, feats, W)
    nc = _get_nc(1)
    res = run_bass_kernel_spmd(nc, in_maps, list(range(NCORES)))
    # out[zt, m=(x_o*ZT+z_o), xt, y] -> [z, x, y]
    parts = []
    for c in range(NCORES):
        o = res.results[c]["out"].reshape(NZT, XT, ZT, NXT, G)
        # dims: zt, x_o, z_o, xt, y  ->  z = zt*ZT+z_o, x = xt*XT+x_o
        o = o.transpose(0, 2, 3, 1, 4).reshape(OZ, OX, G)
        parts.append(o[:G, :XS, :])
    Ofull = np.concatenate(parts, axis=1)          # [z, x, y]
    return Ofull[z, x, y].astype(np.float32).reshape(-1, 1)
